# revision 26
# baseline (speedup 1.0000x reference)
"""Fused FBP (ramp-filter + backprojection + flip + resize + crop) Trainium2 kernel.

The whole reference pipeline is linear in the input sinogram, so it folds into a
single constant matrix T of shape (A*DET, W*W) = (20736, 9216):

    out[n, p] = sum_k x_flat[n, k] * T[k, p]

T has a 4-fold exact symmetry (verified numerically to ~1e-5 of max):
  angle mirror:    T[(215-i, d)]    = mirror_x(T[(i, d)])        (i < 108)
  detector mirror: T[(i, 95-d)]     = rot180(T[(i, d)])          (d < 48)
so only the (i < 108, d < 48) quarter of T is streamed. Four weight sets
accumulate against each streamed tile:

    A: x[i, d]      -> psumA, normal rhs
    B: x[215-i, d]  -> psumB, normal rhs
    C: x[i, 95-d]   -> psumA, column-reversed rhs  (rot180 on outputs)
    D: x[215-i,95-d]-> psumB, column-reversed rhs
    out = psumA + mirror_x(psumB)

The output-pixel axis is sharded across the 8 cores as y-mirror-closed row sets
L_c = {6c..6c+5} u {90-6c..95-6c} (so rot180 of a shard is exactly its column
reversal). T is built on host once (numpy) and streamed from HBM in bf16; x is
replicated in bf16; accumulation is fp32 in PSUM.
"""

import numpy as np
import ml_dtypes

N_ANGLES = 216
DET = 96
WIDTH = 96
UPSAMPLE = 1.8
PAD = 256

SLICES = 96                    # 2*1*48 sinogram slices
K = N_ANGLES * DET             # 20736 full contraction length
P_TOTAL = WIDTH * WIDTH        # 9216 output pixels per slice
NCORES = 8
PSH = P_TOTAL // NCORES        # 1152 output pixels per core
A_HALF = N_ANGLES // 2         # 108
D_HALF = DET // 2              # 48
KQ = A_HALF * D_HALF           # 5184 quarter rows
KCQ = (KQ + 127) // 128        # 41 k-chunks (last one zero-padded)
KQP = KCQ * 128                # 5248 padded rows
GROUPS = [1] * 5 + [3] * 12    # k-chunks per DMA group, sums to 41
RING = 8                       # tt ring depth
MP = 128                       # slice dim padded 96->128 so FWL (fast weight
                               # load) triggers: it requires NumWeights==128

_cache = {}


def _row_set(c):
    """y rows owned by core c, ordered so rot180(shard) == reversed columns."""
    return list(range(6 * c, 6 * c + 6)) + list(range(90 - 6 * c, 96 - 6 * c))


def _build_T_quarter():
    """T rows for angles i<108, detector d<48: (5184, 9216) float32."""
    # --- ramp filter as a circular-convolution matrix (filt = sino @ F) ---
    n = np.concatenate((np.arange(1, PAD // 2 + 1, 2), np.arange(PAD // 2 - 1, 0, -2)))
    f = np.zeros(PAD)
    f[0] = 0.25
    f[1::2] = -1.0 / (np.pi * n) ** 2
    full = 2.0 * np.real(np.fft.fft(f))
    ramp_bins = full[: PAD // 2 + 1].astype(np.float32).astype(np.float64)
    kern = np.fft.irfft(ramp_bins, n=PAD)
    s = np.pi / (2.0 * N_ANGLES)
    jj = np.arange(DET)[:, None]
    ii = np.arange(D_HALF)[None, :]
    F = (s * kern[(ii - jj) % PAD]).astype(np.float32)       # (DET j_in, 48 d_out)

    # --- backprojection weights as hat functions: W[a,d,p] = relu(1-|d-uc|)*inb ---
    angles = np.linspace(0.0, np.pi, N_ANGLES).astype(np.float32).astype(np.float64)[:A_HALF]
    grid = np.arange(WIDTH) - (WIDTH - 1) / 2.0
    ys, xs = np.meshgrid(grid, grid, indexing="ij")
    t = xs[None] * np.cos(angles)[:, None, None] + ys[None] * np.sin(angles)[:, None, None]
    u = t + (DET - 1) / 2.0                                  # (108, W, W)
    inb = ((u >= 0.0) & (u <= DET - 1)).astype(np.float32)
    uc = np.clip(u, 0.0, DET - 1).astype(np.float32)
    uc_flat = uc.reshape(A_HALF, P_TOTAL) * inb.reshape(A_HALF, P_TOTAL)
    inb_flat = inb.reshape(A_HALF, P_TOTAL)
    d = np.arange(DET, dtype=np.float32)
    T1 = np.empty((A_HALF, D_HALF, P_TOTAL), dtype=np.float32)
    for a in range(A_HALF):
        Wa = np.maximum(0.0, 1.0 - np.abs(d[:, None] - uc_flat[a][None, :])) * inb_flat[a][None, :]
        T1[a] = F.T @ Wa                                     # rows j = filtered-d 0..47

    # --- flip both spatial dims ---
    T1 = T1.reshape(A_HALF, D_HALF, WIDTH, WIDTH)[:, :, ::-1, ::-1]

    # --- upsample(1.8, linear, align_corners=False) + center-crop as one matrix ---
    up = int(WIDTH * UPSAMPLE)
    crop = (up - WIDTH) // 2
    coords = (np.arange(up) + 0.5) * (WIDTH / up) - 0.5
    coords = np.clip(coords, 0.0, WIDTH - 1)
    i0 = np.floor(coords).astype(np.int64)
    i1 = np.minimum(i0 + 1, WIDTH - 1)
    w = (coords - i0).astype(np.float32)
    C = np.zeros((WIDTH, up), dtype=np.float32)
    np.add.at(C, (i0, np.arange(up)), 1.0 - w)
    np.add.at(C, (i1, np.arange(up)), w)
    C = np.ascontiguousarray(C[:, crop : crop + WIDTH])      # (y in, Y out)

    T2 = np.tensordot(T1, C, axes=([2], [0]))                # (108, 48, X, Y)
    T2 = np.tensordot(T2, C, axes=([2], [0]))                # (108, 48, Y, X)
    return T2.reshape(KQ, P_TOTAL)


def _build_bass():
    import concourse.bass as bass
    import concourse.mybir as mybir
    from contextlib import ExitStack

    NG = len(GROUPS)
    GMAX = max(GROUPS)
    g_start = [sum(GROUPS[:i]) for i in range(NG)]

    nc = bass.Bass()
    xt = nc.declare_dram_parameter("xt", [128, 4 * KCQ * MP], mybir.dt.bfloat16, isOutput=False)
    tsh = nc.declare_dram_parameter("tsh", [KCQ, 128, PSH], mybir.dt.bfloat16, isOutput=False)
    out = nc.declare_dram_parameter("out", [SLICES, PSH], mybir.dt.float32, isOutput=True)

    with ExitStack() as stack:
        xt_sb = stack.enter_context(nc.sbuf_tensor([128, 4 * KCQ * MP], mybir.dt.bfloat16))
        tt = stack.enter_context(nc.sbuf_tensor([128, RING, GMAX, PSH], mybir.dt.bfloat16))
        scratch = stack.enter_context(nc.sbuf_tensor([128, 512], mybir.dt.bfloat16))
        psumA = stack.enter_context(nc.psum_tensor([MP, PSH], mybir.dt.float32))
        psumB = stack.enter_context(nc.psum_tensor([MP, PSH], mybir.dt.float32))
        psumW = stack.enter_context(nc.psum_tensor([128, 512], mybir.dt.float32))
        o_sb = stack.enter_context(nc.sbuf_tensor([SLICES, PSH], mybir.dt.float32))
        # one DMA in flight per semaphore: in-flight DMA completions on a ring
        # interleave per-SDMA-engine, so a shared counter cannot order them
        dma_sems = [stack.enter_context(nc.semaphore(f"dma_sem{b}")) for b in range(RING)]
        pe_sem = stack.enter_context(nc.semaphore("pe_sem"))
        copy_sem = stack.enter_context(nc.semaphore("copy_sem"))
        out_sem = stack.enter_context(nc.semaphore("out_sem"))
        warm_sem = stack.enter_context(nc.semaphore("warm_sem"))
        block = stack.enter_context(nc.Block())

        # xt upload in pieces so PE can start before the whole 4MB lands;
        # layout interleaves the A/B/C/D sets per chunk (consumption order)
        XP = 8
        piece = 4 * KCQ * MP // XP
        assert piece * XP == 4 * KCQ * MP
        xt_sems = [stack.enter_context(nc.semaphore(f"xt_sem{i}")) for i in range(XP)]

        @block.scalar
        def _(scalar):
            for i in range(XP):
                scalar.dma_start(
                    out=xt_sb[:, i * piece : (i + 1) * piece],
                    in_=xt[:, i * piece : (i + 1) * piece],
                ).then_inc(xt_sems[i], 16)

        @block.sync
        def _(s):
            for g in range(NG):
                if g >= RING:
                    s.wait_ge(pe_sem, g - RING + 1)
                k0, gl = g_start[g], GROUPS[g]
                s.dma_start(
                    out=tt[:, g % RING, 0:gl],
                    in_=tsh[k0 : k0 + gl].rearrange("k p n -> p k n"),
                ).then_inc(dma_sems[g % RING], 16)
            # out DMA pipelined per 384-col region behind the DVE epilogue
            for r in range(3):
                s.wait_ge(copy_sem, 2 * (r + 1))
                s.dma_start(
                    out=out[:, r * 384 : (r + 1) * 384],
                    in_=o_sb[:, r * 384 : (r + 1) * 384],
                ).then_inc(out_sem, 16)
            s.wait_ge(out_sem, 48)

        @block.tensor
        def _(te):
            # HAM warm-up while the first T tile is in flight: junk matmuls
            # into a scratch PSUM bank nothing ever reads
            te.wait_ge(warm_sem, 1)
            for _ in range(5):
                nc.tensor.matmul(
                    psumW[:, :], scratch[:, 0:128], scratch[:, :], start=True, stop=True
                )
            for g in range(NG):
                te.wait_ge(dma_sems[g % RING], (g // RING + 1) * 16)
                k0, gl = g_start[g], GROUPS[g]
                hi_col = (k0 + gl) * 4 * MP
                need = min(XP, (hi_col + piece - 1) // piece)
                te.wait_ge(xt_sems[need - 1], 16)
                last = None
                for j in range(gl):
                    k = k0 + j
                    w = [
                        xt_sb[:, (4 * k + q) * MP : (4 * k + q + 1) * MP]
                        for q in range(4)
                    ]
                    rhs_fwd = tt[:, g % RING, j]
                    rhs_rev = tt[:, g % RING, j, ::-1]
                    for lhsT, psum, rhs in (
                        (w[0], psumA, rhs_fwd),
                        (w[1], psumB, rhs_fwd),
                        (w[2], psumA, rhs_rev),
                        (w[3], psumB, rhs_rev),
                    ):
                        # psumA is first written by set A (w[0]) and last by set
                        # C (w[2]); psumB first by B (w[1]), last by D (w[3])
                        first = k == 0 and (lhsT is w[0] or lhsT is w[1])
                        final = k == KCQ - 1 and (lhsT is w[2] or lhsT is w[3])
                        for off, nn in ((0, 512), (512, 512), (1024, 128)):
                            last = nc.tensor.matmul(
                                psum[:, off : off + nn],
                                lhsT,
                                rhs[:, off : off + nn],
                                start=first,
                                stop=final,
                                skip_group_check=True,
                            )
                last.then_inc(pe_sem, 1)

        @block.vector
        def _(v):
            nc.vector.memset(scratch[:, :], 0).then_inc(warm_sem, 1)
            v.wait_ge(pe_sem, NG)
            # out = A + mirror_x(B): B viewed as (96, 12 rows, 96 x) with x
            # reversed; pipelined in 3 regions of 4 output rows so the out
            # DMA overlaps. DVE may read only one PSUM operand per op.
            psumB_r = psumB[0:SLICES, :].rearrange("p (r x) -> p r x", x=WIDTH)
            psumA_r = psumA[0:SLICES, :].rearrange("p (r x) -> p r x", x=WIDTH)
            o_r = o_sb.rearrange("p (r x) -> p r x", x=WIDTH)
            for r in range(3):
                rows = slice(4 * r, 4 * (r + 1))
                nc.vector.tensor_copy(
                    o_r[:, rows], psumB_r[:, rows, ::-1]
                ).then_inc(copy_sem, 1)
                v.wait_ge(copy_sem, 2 * r + 1)
                nc.vector.tensor_add(
                    o_r[:, rows], o_r[:, rows], psumA_r[:, rows]
                ).then_inc(copy_sem, 1)

    return nc


def _get_state():
    if "state" not in _cache:
        T = _build_T_quarter()
        t_bf = np.zeros((KQP, P_TOTAL), dtype=ml_dtypes.bfloat16)
        t_bf[:KQ] = T.astype(ml_dtypes.bfloat16)
        t_bf = t_bf.reshape(KCQ, 128, P_TOTAL)
        shards = []
        for c in range(NCORES):
            cols = np.array(
                [y * WIDTH + x for y in _row_set(c) for x in range(WIDTH)], dtype=np.int64
            )
            shards.append(np.ascontiguousarray(t_bf[:, :, cols]))
        _cache["state"] = (shards, _build_bass())
    return _cache["state"]


def _pack_lhsT(x_cols):
    """(SLICES, KQ) -> (128, KCQ, MP): zero-padded to KQP rows and MP slices."""
    xp = np.zeros((MP, KQP), dtype=x_cols.dtype)
    xp[:SLICES, :KQ] = x_cols
    return xp.T.reshape(KCQ, 128, MP).transpose(1, 0, 2)


def _make_xt(x_flat):
    v = x_flat.reshape(SLICES, N_ANGLES, DET)
    vr = v[:, ::-1]                                     # angle 215-i at block i
    xA = v[:, :A_HALF, :D_HALF].reshape(SLICES, KQ)
    xB = vr[:, :A_HALF, :D_HALF].reshape(SLICES, KQ)
    xC = v[:, :A_HALF, ::-1][:, :, :D_HALF].reshape(SLICES, KQ)   # d -> 95-d
    xD = vr[:, :A_HALF, ::-1][:, :, :D_HALF].reshape(SLICES, KQ)
    packs = [_pack_lhsT(q) for q in (xA, xB, xC, xD)]
    return np.ascontiguousarray(
        np.stack(packs, axis=2).reshape(128, 4 * KCQ * MP)
    ).astype(ml_dtypes.bfloat16)


def kernel(x, encoder_input_dims=None, decoder_target_shape=None, _want_perf=False):
    from concourse.bass_utils import run_bass_kernel_spmd

    shards, nc = _get_state()
    x = np.asarray(x, dtype=np.float32)
    xt_host = _make_xt(x.reshape(SLICES, K))
    in_maps = [{"xt": xt_host, "tsh": shards[c]} for c in range(NCORES)]
    res = run_bass_kernel_spmd(
        nc, in_maps, core_ids=list(range(NCORES)), trace=_want_perf
    )
    out = np.empty((SLICES, WIDTH, WIDTH), dtype=np.float32)
    for c in range(NCORES):
        r = res.results[c]["out"]
        for t, y in enumerate(_row_set(c)):
            out[:, y, :] = r[:, t * WIDTH : (t + 1) * WIDTH]
    out = out.reshape(2, 1, 48, WIDTH, WIDTH)
    if _want_perf:
        return out, res
    return out
